# revision 22
# baseline (speedup 1.0000x reference)
"""TRN2 Bass kernel for DenseDilatedKnnGraph (B=4, C=64, N=4096, k=9, dilation=2).

Algorithm v2 (tournament-tree candidate selection + exact host rescore)
----------------------------------------------------------------------
reference: xt (B,N,C); dist(i,j) = |xi|^2 - 2<xi,xj> + |xj|^2; nn_idx = top-18
of -dist per row (stable, lowest-index tie-break); output nn_idx[..., ::2] plus
a center-index row -> (2, B, N, 9) int32.

Per-row ordering of -dist equals the ordering of s_ij = 2<xi,xj> - |xj|^2.
The device computes an APPROXIMATE s~ (single fp16 matmul, error ~0.01) that
is only used to SELECT candidate columns; the host rescores candidates in
fp64, so device values never need to be exact.

Device (per core, SPMD over 8 cores; core = (batch, query-half)):
  - v~ = ALPHA*s~ + beta_q via ONE fp16 K=128 matmul into PSUM fp32:
      stationary [ALPHA*2x_q (64); 1; 1; beta_q/2; beta_q/2; 0...],
      moving    [x_c (64); -ALPHA*|x_c|^2/2 (x2); 1; 1; junk]
    beta_q = VCENTER + ALPHA*(dist_est_q - |x_q|^2) places each row's
    nearest-neighbor region near VCENTER..VCENTER+10*ALPHA on a uint16 grid
    (dist_est_q = sampled-min distance estimate, host-computed). The self
    match (dist 0) saturates to 65535, far columns clamp to 0 -- both are
    handled by the host detectors. 128-query tiles, 512-wide PSUM chunks,
    [128,2048] PSUM buffers x2.
  - Tournament max tree in uint16 (the saturating fp32->uint16 cast is
    monotone, so it commutes with max and the tree equals uint16-cast maxima
    exactly; the 2-byte dtype runs every tree level at the DVE 2x rate):
      stage (scalar): each 1024-col PSUM quarter-buffer is cast+copied to
                   SBUF uint16 on the scalar engine (a TensorTensor may read
                   at most one PSUM operand, and only an all-2-byte op gets
                   the DVE 2x mode, so values are staged as uint16 first;
                   4 quarter-buffers rotate through a PE -> scalar -> DVE
                   chain with fine granularity)
      L1 (DVE):    T1[j] = max(cs0[j], cs1[j]) per half (pairs j, j+1024)
      L2..L4 (DVE): T2[j] = max(T1[j], T1[j+1024]); T3[j] = max(T2[j],
                   T2[j+512]); T4[j] = max(T3[j], T3[j+256]) -> T4 256 wide,
                   T4[j] covers original columns {j + 256k, k=0..15}.
                   (TensorTensor does not lower on the Pool/GpSimd engine,
                   so the whole tree lives on DVE.)
  - DVE max8 (top-8 values per T4 group) + max_index (first-occurrence local
    slot) on the NARROW T4 only: NG groups over [0,256).
  - DMA out: local slot indices L (128 x NG*8 uint16). Values stay on device.

Host: slot -> T4 position p -> 16 candidate columns {p + 256k}. Rescore all
48*16 = 768 candidates per row exactly (fp64 BLAS), stable top-18 by
(value desc, col asc) == jax.lax.top_k ordering. Soundness: a true top-18
member's T4 slot is either selected (margin ~100 sigma) or its row is flagged
for full exact repair by
  (a) margin rule: some group's 8 slot-winners all within MARGIN=0.5 of the
      row's 18th-best (device noise incl. uint16 quantization < 0.15 worst
      case), or
  (b) duplicate-slot rule: the same slot appears twice in a group (hw
      max_index actually returns sequential distinct indices for tied
      values, so this never fires on hw; kept as a cheap safety net).
Flagged rows (~8%) get a full 4096-wide fp64 recompute on host.
"""

import numpy as np

import concourse.bacc as bacc
import concourse.mybir as mybir
import concourse.tile as tile
from concourse.bass_utils import run_bass_kernel_spmd

# Problem constants (hardcoded per harness contract).
B = 4
C = 64
N = 4096
K = 9
DILATION = 2
K_EFF = K * DILATION      # 18
P = 128                   # partitions / queries per tile
KM = 128                  # matmul contraction rows (K=66 measured ~10% slower
                          # per matmul than K=128, so keep 128)
N_CORES = 8
QROWS = (B * N) // N_CORES          # 2048 query rows per core
N_TILES = QROWS // P                # 16 tiles per core

TOPW = 256                # tournament top level width (each slot = 16 columns)
STRIDE = N // TOPW        # 16: original columns of top-level slot p: p + 256k
# max8 group bounds over T4 [0,256). 5 groups of ~51 slots = ~820 original
# columns each; P(a group truly holds >=8 of the top-18) ~ 7.8% of rows,
# which (plus the detector margin) sets the host repair rate.
GB3 = (0, 51, 102, 154, 205, 256)
NG = len(GB3) - 1
UW = NG * 8               # 48 candidate slots per row
MARGIN = 0.5              # hazard detector band (device noise <= ~0.12)
ALPHA0 = 1000.0           # fixed scale baked into the shared -|x_c|^2 rows
VCENTER = 24000.0         # grid value at dist == dist_est
SELF_V = 65100.0          # grid value the self match (dist 0) is pinned to;
                          # per-row alpha_q = (SELF_V-VCENTER)/dist_est makes
                          # v < SELF_V for every dist > 0, so nothing can
                          # wrap past 65535 (hw cast wraps, not saturates)


def _build_program(n_tiles=N_TILES):
    nc = bacc.Bacc(
        "TRN2", target_bir_lowering=False, debug=False, enable_asserts=False
    )
    f32 = mybir.dt.float32
    f16 = mybir.dt.float16
    u16 = mybir.dt.uint16
    nq = n_tiles * P
    lhs_a = nc.dram_tensor("lhs_a", (KM, nq), f16, kind="ExternalInput")
    rhs_a = nc.dram_tensor("rhs_a", (KM, N), f16, kind="ExternalInput")
    l_out = nc.dram_tensor("l_out", (nq, UW), u16, kind="ExternalOutput")
    lhs_a_ap, rhs_a_ap = lhs_a.ap(), rhs_a.ap()
    l_ap = l_out.ap()

    with tile.TileContext(nc) as tc:
        with (
            tc.tile_pool(name="const", bufs=1) as cpool,
            tc.tile_pool(name="psum", bufs=2, space="PSUM") as ppool,
            tc.tile_pool(name="csp", bufs=4) as cspool,
            tc.tile_pool(name="t1p", bufs=3) as t1pool,
            tc.tile_pool(name="t2p", bufs=2) as t2pool,
            tc.tile_pool(name="t3p", bufs=2) as t3pool,
            tc.tile_pool(name="outp", bufs=4) as opool,
        ):
            # dependency-free warm-up matmuls that run during the input-DMA
            # prologue (nudges the PE toward its full-rate p-state)
            prime = cpool.tile([KM, 512], f16)
            nc.gpsimd.memset(prime[:, :], 0.0)
            pps = ppool.tile([P, 2048], f32, tag="ps")
            for _ in range(3):
                nc.tensor.matmul(pps[:, :512], prime[:, :128], prime[:, :],
                                 start=True, stop=True)

            # per-512-column-chunk input tiles: the first matmul only waits
            # for its own chunk, not the whole load
            ra_sb = [
                cpool.tile([KM, 512], f16, name=f"ra{j}", tag=f"ra{j}")
                for j in range(8)
            ]
            la_sb = cpool.tile([KM, nq], f16)
            w0 = min(512, nq)
            nc.sync.dma_start(la_sb[:, 0:w0], lhs_a_ap[:, 0:w0])
            nc.sync.dma_start(ra_sb[0][:, :], rhs_a_ap[:, 0:512])
            for j in range(1, 8):
                nc.sync.dma_start(ra_sb[j][:, :], rhs_a_ap[:, j * 512 : (j + 1) * 512])
            for j in range(512, nq, 512):
                w = min(512, nq - j)
                nc.sync.dma_start(la_sb[:, j : j + w], lhs_a_ap[:, j : j + w])

            for t in range(n_tiles):
                qs = slice(t * P, (t + 1) * P)
                t1 = t1pool.tile([P, 2048], u16, tag="t1")
                for h in range(2):
                    ps = ppool.tile([P, 2048], f32, tag="ps")
                    cs = cspool.tile([P, 2048], u16, tag="cs")
                    for j in range(4):
                        cj = h * 4 + j
                        nc.tensor.matmul(
                            ps[:, j * 512 : (j + 1) * 512],
                            la_sb[:, qs], ra_sb[cj][:, :],
                            start=True, stop=True,
                        )
                    # one merged u16-cast copy per half (fewer fixed overheads)
                    nc.scalar.copy(cs[:, :], ps[:, :])
                    nc.vector.tensor_max(
                        t1[:, h * 1024 : (h + 1) * 1024],
                        cs[:, 0:1024], cs[:, 1024:2048],
                    )

                # L2..L4 on DVE (TensorTensor does not lower on gpsimd)
                t2 = t2pool.tile([P, 1024], u16, tag="t2")
                nc.vector.tensor_max(t2[:, :], t1[:, 0:1024], t1[:, 1024:2048])
                t3 = t3pool.tile([P, 512], u16, tag="t3")
                nc.vector.tensor_max(t3[:, :], t2[:, 0:512], t2[:, 512:1024])
                t4 = t3pool.tile([P, TOPW], u16, tag="t4")
                nc.vector.tensor_max(t4[:, :], t3[:, 0:256], t3[:, 256:512])

                # top-8 per group + first-occurrence local slot index
                u = opool.tile([P, UW], u16, tag="u")
                l = opool.tile([P, UW], u16, tag="l")
                for g in range(NG):
                    nc.vector.max(
                        out=u[:, g * 8 : (g + 1) * 8],
                        in_=t4[:, GB3[g] : GB3[g + 1]],
                    )
                for g in range(NG):
                    nc.vector.max_index(
                        out=l[:, g * 8 : (g + 1) * 8],
                        in_max=u[:, g * 8 : (g + 1) * 8],
                        in_values=t4[:, GB3[g] : GB3[g + 1]],
                    )

                rs = slice(t * P, (t + 1) * P)
                nc.sync.dma_start(l_ap[rs, :], l[:])
    nc.compile()
    return nc


_SAMPLE_COLS = np.arange(11, N, 21)[:192]         # 192 fixed probe columns


def _prep_core_inputs(X, core):
    """X: (B, N, C) fp32. Returns input map for one core."""
    b, h = divmod(core, N_CORES // B)
    Xb = X[b]
    xsq = np.sum(Xb * Xb, axis=1, dtype=np.float32)
    ch = Xb.T.astype(np.float16)                  # (C, N)
    half_s = (-(ALPHA0 * 0.5) * xsq).astype(np.float16)
    rhs_a = np.zeros((KM, N), np.float16)
    rhs_a[:C] = ch
    rhs_a[C] = half_s
    rhs_a[C + 1] = half_s
    rhs_a[C + 2 : C + 4] = 1.0

    Q = Xb[h * QROWS : (h + 1) * QROWS]           # (QROWS, C)
    qsq = xsq[h * QROWS : (h + 1) * QROWS]
    # sampled nearest-distance estimate per query (approximate is fine: only
    # resolution depends on it, never clamp-soundness). The diagonal must be
    # masked: a probe column equal to the query itself gives dist 0 and a
    # garbage estimate.
    dprobe = (
        qsq[:, None]
        + xsq[_SAMPLE_COLS][None, :]
        - 2.0 * (Q @ Xb[_SAMPLE_COLS].T)
    )
    qglob = h * QROWS + np.arange(QROWS)
    dprobe[qglob[:, None] == _SAMPLE_COLS[None, :]] = np.inf
    dsamp = dprobe.min(axis=1)
    dist_est = np.maximum(dsamp, (SELF_V - VCENTER) / 1024.0 + 0.1)
    alpha_q = (SELF_V - VCENTER) / dist_est       # per-row scale, <= 1024
    beta = SELF_V - alpha_q * qsq                 # v_self == SELF_V exactly
    qh = ((2.0 * alpha_q)[None, :] * Q.T).astype(np.float16)
    lhs_a = np.zeros((KM, QROWS), np.float16)
    lhs_a[:C] = qh
    lhs_a[C : C + 2] = (alpha_q / ALPHA0).astype(np.float16)
    lhs_a[C + 2 : C + 4] = (0.5 * beta).astype(np.float16)
    return {"lhs_a": lhs_a, "rhs_a": rhs_a}


# slot s -> group g = s//8; T3 position p = GB3[g] + l[s]; columns p + 512k
_GOFF = np.asarray(GB3[:-1], dtype=np.int64)[np.arange(UW) // 8]   # (48,)
_KOFF = (np.arange(STRIDE, dtype=np.int64) * TOPW)                 # (16,)


def _merge_core(L, Xb64, xsq64, q0):
    """L: (R, 48) uint16 local slot indices for queries q0..q0+R-1 of batch b.
    Returns (idx (R,18) int64, flagged-row mask (R,))."""
    R = L.shape[0]
    Ppos = L.astype(np.int64) + _GOFF[None, :]               # (R, 48) in [0,TOPW)
    cols = Ppos[:, :, None] + _KOFF[None, None, :]           # (R, 48, 16)

    # duplicate-slot rule: same T3 position twice within a group
    ps = np.sort(Ppos.reshape(R, NG, 8), axis=2)
    dup = (np.diff(ps, axis=2) == 0).any(axis=(1, 2))

    idx = np.empty((R, K_EFF), np.int64)
    flag = np.empty(R, bool)
    CH = 512
    NC_ = UW * STRIDE
    for c0 in range(0, R, CH):
        c1 = min(c0 + CH, R)
        cc = cols[c0:c1].reshape(c1 - c0, NC_)                # (r, 768)
        # the self column always belongs to the true top-18 (dist 0) but its
        # on-device value wraps mod 2^16 -- inject it unconditionally
        selfc = np.arange(q0 + c0, q0 + c1, dtype=np.int64)[:, None]
        cc = np.concatenate([cc, selfc], axis=1)              # (r, 769)
        g = Xb64[cc]                                          # (r, 769, 64)
        xq = Xb64[q0 + c0 : q0 + c1]                          # (r, 64)
        vals = 2.0 * np.matmul(g, xq[:, :, None])[:, :, 0]    # (r, 769)
        vals -= xsq64[cc]

        # margin rule: per-slot winner, per-group min of the 8 winners
        # (device slots only -- exclude the injected self column)
        w = vals[:, :NC_].reshape(c1 - c0, UW, STRIDE).max(axis=2)
        gmin = w.reshape(c1 - c0, NG, 8).min(axis=2)          # (r, NG)
        t18 = np.partition(vals, vals.shape[1] - K_EFF, axis=1)[
            :, vals.shape[1] - K_EFF
        ]
        flag[c0:c1] = (gmin >= (t18[:, None] - MARGIN)).any(axis=1)

        # the self col may duplicate a device candidate: mask the device copy
        dupself = cc[:, :NC_] == selfc
        vals[:, :NC_][dupself] = -np.inf

        # stable top-18 by (value desc, col asc): sort cols ascending first
        corder = np.argsort(cc, axis=1, kind="stable")
        fc_s = np.take_along_axis(cc, corder, axis=1)
        va_s = np.take_along_axis(vals, corder, axis=1)
        vorder = np.argsort(-va_s, axis=1, kind="stable")[:, :K_EFF]
        idx[c0:c1] = np.take_along_axis(fc_s, vorder, axis=1)
    return idx, (flag | dup)


_NC_CACHE = {}


def kernel(x: np.ndarray) -> np.ndarray:
    x = np.asarray(x)
    assert x.shape == (B, C, N, 1), x.shape
    X = np.ascontiguousarray(np.transpose(x[..., 0], (0, 2, 1)))  # (B, N, C)

    if N_TILES not in _NC_CACHE:
        _NC_CACHE[N_TILES] = _build_program(N_TILES)
    nc = _NC_CACHE[N_TILES]

    in_maps = [_prep_core_inputs(X, c) for c in range(N_CORES)]
    res = run_bass_kernel_spmd(nc, in_maps, core_ids=list(range(N_CORES)))

    X64 = X.astype(np.float64)
    xsq64 = np.einsum("bnc,bnc->bn", X64, X64)

    nn_idx = np.empty((B, N, K_EFF), np.int64)
    bad_rows = [[] for _ in range(B)]
    for core in range(N_CORES):
        b, h = divmod(core, N_CORES // B)
        r = res.results[core]
        idx, bad = _merge_core(r["l_out"], X64[b], xsq64[b], h * QROWS)
        nn_idx[b, h * QROWS : (h + 1) * QROWS] = idx
        if bad.any():
            bad_rows[b].extend((h * QROWS + np.nonzero(bad)[0]).tolist())

    # full exact recompute of flagged rows
    for b in range(B):
        if not bad_rows[b]:
            continue
        rows = np.asarray(sorted(bad_rows[b]))
        S = 2.0 * (X64[b, rows] @ X64[b].T) - xsq64[b][None, :]
        order = np.argsort(-S, axis=1, kind="stable")
        nn_idx[b, rows] = order[:, :K_EFF]

    nn_dil = nn_idx[:, :, ::DILATION]                       # (B, N, 9)
    center = np.broadcast_to(np.arange(N)[None, :, None], nn_dil.shape)
    out = np.stack((nn_dil, center), axis=0).astype(np.int32)
    return out


# revision 23
# speedup vs baseline: 1.1673x; 1.1673x over previous
"""TRN2 Bass kernel for DenseDilatedKnnGraph (B=4, C=64, N=4096, k=9, dilation=2).

Algorithm v2 (tournament-tree candidate selection + exact host rescore)
----------------------------------------------------------------------
reference: xt (B,N,C); dist(i,j) = |xi|^2 - 2<xi,xj> + |xj|^2; nn_idx = top-18
of -dist per row (stable, lowest-index tie-break); output nn_idx[..., ::2] plus
a center-index row -> (2, B, N, 9) int32.

Per-row ordering of -dist equals the ordering of s_ij = 2<xi,xj> - |xj|^2.
The device computes an APPROXIMATE s~ (single fp16 matmul, error ~0.01) that
is only used to SELECT candidate columns; the host rescores candidates in
fp64, so device values never need to be exact.

Device (per core, SPMD over 8 cores; core = (batch, query-half)):
  - v~ = ALPHA*s~ + beta_q via ONE fp16 K=128 matmul into PSUM fp32:
      stationary [ALPHA*2x_q (64); 1; 1; beta_q/2; beta_q/2; 0...],
      moving    [x_c (64); -ALPHA*|x_c|^2/2 (x2); 1; 1; junk]
    beta_q = VCENTER + ALPHA*(dist_est_q - |x_q|^2) places each row's
    nearest-neighbor region near VCENTER..VCENTER+10*ALPHA on a uint16 grid
    (dist_est_q = sampled-min distance estimate, host-computed). The self
    match (dist 0) saturates to 65535, far columns clamp to 0 -- both are
    handled by the host detectors. 128-query tiles, 512-wide PSUM chunks,
    [128,2048] PSUM buffers x2.
  - Tournament max tree in uint16 (the saturating fp32->uint16 cast is
    monotone, so it commutes with max and the tree equals uint16-cast maxima
    exactly; the 2-byte dtype runs every tree level at the DVE 2x rate):
      stage (scalar): each 1024-col PSUM quarter-buffer is cast+copied to
                   SBUF uint16 on the scalar engine (a TensorTensor may read
                   at most one PSUM operand, and only an all-2-byte op gets
                   the DVE 2x mode, so values are staged as uint16 first;
                   4 quarter-buffers rotate through a PE -> scalar -> DVE
                   chain with fine granularity)
      L1 (DVE):    T1[j] = max(cs0[j], cs1[j]) per half (pairs j, j+1024)
      L2..L4 (DVE): T2[j] = max(T1[j], T1[j+1024]); T3[j] = max(T2[j],
                   T2[j+512]); T4[j] = max(T3[j], T3[j+256]) -> T4 256 wide,
                   T4[j] covers original columns {j + 256k, k=0..15}.
                   (TensorTensor does not lower on the Pool/GpSimd engine,
                   so the whole tree lives on DVE.)
  - DVE max8 (top-8 values per T4 group) + max_index (first-occurrence local
    slot) on the NARROW T4 only: NG groups over [0,256).
  - DMA out: local slot indices L (128 x NG*8 uint16). Values stay on device.

Host: slot -> T4 position p -> 16 candidate columns {p + 256k}. Rescore all
48*16 = 768 candidates per row exactly (fp64 BLAS), stable top-18 by
(value desc, col asc) == jax.lax.top_k ordering. Soundness: a true top-18
member's T4 slot is either selected (margin ~100 sigma) or its row is flagged
for full exact repair by
  (a) margin rule: some group's 8 slot-winners all within MARGIN=0.5 of the
      row's 18th-best (device noise incl. uint16 quantization < 0.15 worst
      case), or
  (b) duplicate-slot rule: the same slot appears twice in a group (hw
      max_index actually returns sequential distinct indices for tied
      values, so this never fires on hw; kept as a cheap safety net).
Flagged rows (~8%) get a full 4096-wide fp64 recompute on host.
"""

import numpy as np

import concourse.bacc as bacc
import concourse.mybir as mybir
import concourse.tile as tile
from concourse.bass_utils import run_bass_kernel_spmd

# Problem constants (hardcoded per harness contract).
B = 4
C = 64
N = 4096
K = 9
DILATION = 2
K_EFF = K * DILATION      # 18
P = 128                   # partitions / queries per tile
KM = 128                  # matmul contraction rows (K=66 measured ~10% slower
                          # per matmul than K=128, so keep 128)
N_CORES = 8
QROWS = (B * N) // N_CORES          # 2048 query rows per core
N_TILES = QROWS // P                # 16 tiles per core

TOPW = 256                # tournament top level width (each slot = 16 columns)
STRIDE = N // TOPW        # 16: original columns of top-level slot p: p + 256k
# max8 group bounds over T4 [0,256). 5 groups of ~51 slots = ~820 original
# columns each; P(a group truly holds >=8 of the top-18) ~ 7.8% of rows,
# which (plus the detector margin) sets the host repair rate.
GB3 = (0, 51, 102, 154, 205, 256)
NG = len(GB3) - 1
UW = NG * 8               # 48 candidate slots per row
MARGIN = 0.5              # hazard detector band (device noise <= ~0.12)
ALPHA0 = 1000.0           # fixed scale baked into the shared -|x_c|^2 rows
VCENTER = 24000.0         # grid value at dist == dist_est
SELF_V = 65100.0          # grid value the self match (dist 0) is pinned to;
                          # per-row alpha_q = (SELF_V-VCENTER)/dist_est makes
                          # v < SELF_V for every dist > 0, so nothing can
                          # wrap past 65535 (hw cast wraps, not saturates)


def _build_program(n_tiles=N_TILES):
    nc = bacc.Bacc(
        "TRN2", target_bir_lowering=False, debug=False, enable_asserts=False
    )
    f32 = mybir.dt.float32
    f16 = mybir.dt.float16
    u16 = mybir.dt.uint16
    nq = n_tiles * P
    lhs_a = nc.dram_tensor("lhs_a", (KM, nq), f16, kind="ExternalInput")
    rhs_a = nc.dram_tensor("rhs_a", (KM, N), f16, kind="ExternalInput")
    l_out = nc.dram_tensor("l_out", (nq, UW), u16, kind="ExternalOutput")
    lhs_a_ap, rhs_a_ap = lhs_a.ap(), rhs_a.ap()
    l_ap = l_out.ap()

    with tile.TileContext(nc) as tc:
        with (
            tc.tile_pool(name="const", bufs=1) as cpool,
            tc.tile_pool(name="psum", bufs=2, space="PSUM") as ppool,
            tc.tile_pool(name="csp", bufs=5) as cspool,
            tc.tile_pool(name="t1p", bufs=4) as t1pool,
            tc.tile_pool(name="t2p", bufs=3) as t2pool,
            tc.tile_pool(name="t3p", bufs=4) as t3pool,
            tc.tile_pool(name="outp", bufs=8) as opool,
        ):
            # dependency-free warm-up matmuls that run during the input-DMA
            # prologue (nudges the PE toward its full-rate p-state)
            prime = cpool.tile([KM, 512], f16)
            nc.gpsimd.memset(prime[:, :], 0.0)
            pps = ppool.tile([P, 2048], f32, tag="ps")
            for _ in range(3):
                nc.tensor.matmul(pps[:, :512], prime[:, :128], prime[:, :],
                                 start=True, stop=True)

            # per-512-column-chunk input tiles: the first matmul only waits
            # for its own chunk, not the whole load
            ra_sb = [
                cpool.tile([KM, 512], f16, name=f"ra{j}", tag=f"ra{j}")
                for j in range(8)
            ]
            la_sb = cpool.tile([KM, nq], f16)
            w0 = min(512, nq)
            nc.sync.dma_start(la_sb[:, 0:w0], lhs_a_ap[:, 0:w0])
            nc.sync.dma_start(ra_sb[0][:, :], rhs_a_ap[:, 0:512])
            for j in range(1, 8):
                nc.sync.dma_start(ra_sb[j][:, :], rhs_a_ap[:, j * 512 : (j + 1) * 512])
            for j in range(512, nq, 512):
                w = min(512, nq - j)
                nc.sync.dma_start(la_sb[:, j : j + w], lhs_a_ap[:, j : j + w])

            for t in range(n_tiles):
                qs = slice(t * P, (t + 1) * P)
                t1 = t1pool.tile([P, 2048], u16, tag="t1")
                for h in range(2):
                    ps = ppool.tile([P, 2048], f32, tag="ps")
                    cs = cspool.tile([P, 2048], u16, tag="cs")
                    for j in range(4):
                        cj = h * 4 + j
                        nc.tensor.matmul(
                            ps[:, j * 512 : (j + 1) * 512],
                            la_sb[:, qs], ra_sb[cj][:, :],
                            start=True, stop=True,
                        )
                    # one merged u16-cast copy per half (fewer fixed overheads)
                    nc.scalar.copy(cs[:, :], ps[:, :])
                    nc.vector.tensor_max(
                        t1[:, h * 1024 : (h + 1) * 1024],
                        cs[:, 0:1024], cs[:, 1024:2048],
                    )

                # L2..L4 on DVE (TensorTensor does not lower on gpsimd)
                t2 = t2pool.tile([P, 1024], u16, tag="t2")
                nc.vector.tensor_max(t2[:, :], t1[:, 0:1024], t1[:, 1024:2048])
                t3 = t3pool.tile([P, 512], u16, tag="t3")
                nc.vector.tensor_max(t3[:, :], t2[:, 0:512], t2[:, 512:1024])
                t4 = t3pool.tile([P, TOPW], u16, tag="t4")
                nc.vector.tensor_max(t4[:, :], t3[:, 0:256], t3[:, 256:512])

                # top-8 per group + first-occurrence local slot index
                u = opool.tile([P, UW], u16, tag="u")
                l = opool.tile([P, UW], u16, tag="l")
                for g in range(NG):
                    nc.vector.max(
                        out=u[:, g * 8 : (g + 1) * 8],
                        in_=t4[:, GB3[g] : GB3[g + 1]],
                    )
                for g in range(NG):
                    nc.vector.max_index(
                        out=l[:, g * 8 : (g + 1) * 8],
                        in_max=u[:, g * 8 : (g + 1) * 8],
                        in_values=t4[:, GB3[g] : GB3[g + 1]],
                    )

                rs = slice(t * P, (t + 1) * P)
                nc.sync.dma_start(l_ap[rs, :], l[:])
    nc.compile()
    return nc


_SAMPLE_COLS = np.arange(11, N, 21)[:192]         # 192 fixed probe columns


def _prep_core_inputs(X, core):
    """X: (B, N, C) fp32. Returns input map for one core."""
    b, h = divmod(core, N_CORES // B)
    Xb = X[b]
    xsq = np.sum(Xb * Xb, axis=1, dtype=np.float32)
    ch = Xb.T.astype(np.float16)                  # (C, N)
    half_s = (-(ALPHA0 * 0.5) * xsq).astype(np.float16)
    rhs_a = np.zeros((KM, N), np.float16)
    rhs_a[:C] = ch
    rhs_a[C] = half_s
    rhs_a[C + 1] = half_s
    rhs_a[C + 2 : C + 4] = 1.0

    Q = Xb[h * QROWS : (h + 1) * QROWS]           # (QROWS, C)
    qsq = xsq[h * QROWS : (h + 1) * QROWS]
    # sampled nearest-distance estimate per query (approximate is fine: only
    # resolution depends on it, never clamp-soundness). The diagonal must be
    # masked: a probe column equal to the query itself gives dist 0 and a
    # garbage estimate.
    dprobe = (
        qsq[:, None]
        + xsq[_SAMPLE_COLS][None, :]
        - 2.0 * (Q @ Xb[_SAMPLE_COLS].T)
    )
    qglob = h * QROWS + np.arange(QROWS)
    dprobe[qglob[:, None] == _SAMPLE_COLS[None, :]] = np.inf
    dsamp = dprobe.min(axis=1)
    dist_est = np.maximum(dsamp, (SELF_V - VCENTER) / 1024.0 + 0.1)
    alpha_q = (SELF_V - VCENTER) / dist_est       # per-row scale, <= 1024
    beta = SELF_V - alpha_q * qsq                 # v_self == SELF_V exactly
    qh = ((2.0 * alpha_q)[None, :] * Q.T).astype(np.float16)
    lhs_a = np.zeros((KM, QROWS), np.float16)
    lhs_a[:C] = qh
    lhs_a[C : C + 2] = (alpha_q / ALPHA0).astype(np.float16)
    lhs_a[C + 2 : C + 4] = (0.5 * beta).astype(np.float16)
    return {"lhs_a": lhs_a, "rhs_a": rhs_a}


# slot s -> group g = s//8; T3 position p = GB3[g] + l[s]; columns p + 512k
_GOFF = np.asarray(GB3[:-1], dtype=np.int64)[np.arange(UW) // 8]   # (48,)
_KOFF = (np.arange(STRIDE, dtype=np.int64) * TOPW)                 # (16,)


def _merge_core(L, Xb64, xsq64, q0):
    """L: (R, 48) uint16 local slot indices for queries q0..q0+R-1 of batch b.
    Returns (idx (R,18) int64, flagged-row mask (R,))."""
    R = L.shape[0]
    Ppos = L.astype(np.int64) + _GOFF[None, :]               # (R, 48) in [0,TOPW)
    cols = Ppos[:, :, None] + _KOFF[None, None, :]           # (R, 48, 16)

    # duplicate-slot rule: same T3 position twice within a group
    ps = np.sort(Ppos.reshape(R, NG, 8), axis=2)
    dup = (np.diff(ps, axis=2) == 0).any(axis=(1, 2))

    idx = np.empty((R, K_EFF), np.int64)
    flag = np.empty(R, bool)
    CH = 512
    NC_ = UW * STRIDE
    for c0 in range(0, R, CH):
        c1 = min(c0 + CH, R)
        cc = cols[c0:c1].reshape(c1 - c0, NC_)                # (r, 768)
        # the self column always belongs to the true top-18 (dist 0) but its
        # on-device value wraps mod 2^16 -- inject it unconditionally
        selfc = np.arange(q0 + c0, q0 + c1, dtype=np.int64)[:, None]
        cc = np.concatenate([cc, selfc], axis=1)              # (r, 769)
        g = Xb64[cc]                                          # (r, 769, 64)
        xq = Xb64[q0 + c0 : q0 + c1]                          # (r, 64)
        vals = 2.0 * np.matmul(g, xq[:, :, None])[:, :, 0]    # (r, 769)
        vals -= xsq64[cc]

        # margin rule: per-slot winner, per-group min of the 8 winners
        # (device slots only -- exclude the injected self column)
        w = vals[:, :NC_].reshape(c1 - c0, UW, STRIDE).max(axis=2)
        gmin = w.reshape(c1 - c0, NG, 8).min(axis=2)          # (r, NG)
        t18 = np.partition(vals, vals.shape[1] - K_EFF, axis=1)[
            :, vals.shape[1] - K_EFF
        ]
        flag[c0:c1] = (gmin >= (t18[:, None] - MARGIN)).any(axis=1)

        # the self col may duplicate a device candidate: mask the device copy
        dupself = cc[:, :NC_] == selfc
        vals[:, :NC_][dupself] = -np.inf

        # stable top-18 by (value desc, col asc): sort cols ascending first
        corder = np.argsort(cc, axis=1, kind="stable")
        fc_s = np.take_along_axis(cc, corder, axis=1)
        va_s = np.take_along_axis(vals, corder, axis=1)
        vorder = np.argsort(-va_s, axis=1, kind="stable")[:, :K_EFF]
        idx[c0:c1] = np.take_along_axis(fc_s, vorder, axis=1)
    return idx, (flag | dup)


_NC_CACHE = {}


def kernel(x: np.ndarray) -> np.ndarray:
    x = np.asarray(x)
    assert x.shape == (B, C, N, 1), x.shape
    X = np.ascontiguousarray(np.transpose(x[..., 0], (0, 2, 1)))  # (B, N, C)

    if N_TILES not in _NC_CACHE:
        _NC_CACHE[N_TILES] = _build_program(N_TILES)
    nc = _NC_CACHE[N_TILES]

    in_maps = [_prep_core_inputs(X, c) for c in range(N_CORES)]
    res = run_bass_kernel_spmd(nc, in_maps, core_ids=list(range(N_CORES)))

    X64 = X.astype(np.float64)
    xsq64 = np.einsum("bnc,bnc->bn", X64, X64)

    nn_idx = np.empty((B, N, K_EFF), np.int64)
    bad_rows = [[] for _ in range(B)]
    for core in range(N_CORES):
        b, h = divmod(core, N_CORES // B)
        r = res.results[core]
        idx, bad = _merge_core(r["l_out"], X64[b], xsq64[b], h * QROWS)
        nn_idx[b, h * QROWS : (h + 1) * QROWS] = idx
        if bad.any():
            bad_rows[b].extend((h * QROWS + np.nonzero(bad)[0]).tolist())

    # full exact recompute of flagged rows
    for b in range(B):
        if not bad_rows[b]:
            continue
        rows = np.asarray(sorted(bad_rows[b]))
        S = 2.0 * (X64[b, rows] @ X64[b].T) - xsq64[b][None, :]
        order = np.argsort(-S, axis=1, kind="stable")
        nn_idx[b, rows] = order[:, :K_EFF]

    nn_dil = nn_idx[:, :, ::DILATION]                       # (B, N, 9)
    center = np.broadcast_to(np.arange(N)[None, :, None], nn_dil.shape)
    out = np.stack((nn_dil, center), axis=0).astype(np.int32)
    return out


# revision 24
# speedup vs baseline: 1.1993x; 1.0273x over previous
"""TRN2 Bass kernel for DenseDilatedKnnGraph (B=4, C=64, N=4096, k=9, dilation=2).

Algorithm v2 (tournament-tree candidate selection + exact host rescore)
----------------------------------------------------------------------
reference: xt (B,N,C); dist(i,j) = |xi|^2 - 2<xi,xj> + |xj|^2; nn_idx = top-18
of -dist per row (stable, lowest-index tie-break); output nn_idx[..., ::2] plus
a center-index row -> (2, B, N, 9) int32.

Per-row ordering of -dist equals the ordering of s_ij = 2<xi,xj> - |xj|^2.
The device computes an APPROXIMATE s~ (single fp16 matmul, error ~0.01) that
is only used to SELECT candidate columns; the host rescores candidates in
fp64, so device values never need to be exact.

Device (per core, SPMD over 8 cores; core = (batch, query-half)):
  - v~ = ALPHA*s~ + beta_q via ONE fp16 K=128 matmul into PSUM fp32:
      stationary [ALPHA*2x_q (64); 1; 1; beta_q/2; beta_q/2; 0...],
      moving    [x_c (64); -ALPHA*|x_c|^2/2 (x2); 1; 1; junk]
    beta_q = VCENTER + ALPHA*(dist_est_q - |x_q|^2) places each row's
    nearest-neighbor region near VCENTER..VCENTER+10*ALPHA on a uint16 grid
    (dist_est_q = sampled-min distance estimate, host-computed). The self
    match (dist 0) saturates to 65535, far columns clamp to 0 -- both are
    handled by the host detectors. 128-query tiles, 512-wide PSUM chunks,
    [128,2048] PSUM buffers x2.
  - Tournament max tree in uint16 (the saturating fp32->uint16 cast is
    monotone, so it commutes with max and the tree equals uint16-cast maxima
    exactly; the 2-byte dtype runs every tree level at the DVE 2x rate):
      stage (scalar): each 1024-col PSUM quarter-buffer is cast+copied to
                   SBUF uint16 on the scalar engine (a TensorTensor may read
                   at most one PSUM operand, and only an all-2-byte op gets
                   the DVE 2x mode, so values are staged as uint16 first;
                   4 quarter-buffers rotate through a PE -> scalar -> DVE
                   chain with fine granularity)
      L1 (DVE):    T1[j] = max(cs0[j], cs1[j]) per half (pairs j, j+1024)
      L2..L4 (DVE): T2[j] = max(T1[j], T1[j+1024]); T3[j] = max(T2[j],
                   T2[j+512]); T4[j] = max(T3[j], T3[j+256]) -> T4 256 wide,
                   T4[j] covers original columns {j + 256k, k=0..15}.
                   (TensorTensor does not lower on the Pool/GpSimd engine,
                   so the whole tree lives on DVE.)
  - DVE max8 (top-8 values per T4 group) + max_index (first-occurrence local
    slot) on the NARROW T4 only: NG groups over [0,256).
  - DMA out: local slot indices L (128 x NG*8 uint16). Values stay on device.

Host: slot -> T4 position p -> 16 candidate columns {p + 256k}. Rescore all
48*16 = 768 candidates per row exactly (fp64 BLAS), stable top-18 by
(value desc, col asc) == jax.lax.top_k ordering. Soundness: a true top-18
member's T4 slot is either selected (margin ~100 sigma) or its row is flagged
for full exact repair by
  (a) margin rule: some group's 8 slot-winners all within MARGIN=0.5 of the
      row's 18th-best (device noise incl. uint16 quantization < 0.15 worst
      case), or
  (b) duplicate-slot rule: the same slot appears twice in a group (hw
      max_index actually returns sequential distinct indices for tied
      values, so this never fires on hw; kept as a cheap safety net).
Flagged rows (~8%) get a full 4096-wide fp64 recompute on host.
"""

import numpy as np

import concourse.bacc as bacc
import concourse.mybir as mybir
import concourse.tile as tile
from concourse.bass_utils import run_bass_kernel_spmd

# Problem constants (hardcoded per harness contract).
B = 4
C = 64
N = 4096
K = 9
DILATION = 2
K_EFF = K * DILATION      # 18
P = 128                   # partitions / queries per tile
KM = 128                  # matmul contraction rows (K=66 measured ~10% slower
                          # per matmul than K=128, so keep 128)
N_CORES = 8
QROWS = (B * N) // N_CORES          # 2048 query rows per core
N_TILES = QROWS // P                # 16 tiles per core

TOPW = 256                # tournament top level width (each slot = 16 columns)
STRIDE = N // TOPW        # 16: original columns of top-level slot p: p + 256k
# max8 group bounds over T4 [0,256). 5 groups of ~51 slots = ~820 original
# columns each; P(a group truly holds >=8 of the top-18) ~ 7.8% of rows,
# which (plus the detector margin) sets the host repair rate.
GB3 = (0, 51, 102, 154, 205, 256)
NG = len(GB3) - 1
UW = NG * 8               # 48 candidate slots per row
MARGIN = 0.5              # hazard detector band (device noise <= ~0.12)
ALPHA0 = 1000.0           # fixed scale baked into the shared -|x_c|^2 rows
VCENTER = 24000.0         # grid value at dist == dist_est
SELF_V = 65100.0          # grid value the self match (dist 0) is pinned to;
                          # per-row alpha_q = (SELF_V-VCENTER)/dist_est makes
                          # v < SELF_V for every dist > 0, so nothing can
                          # wrap past 65535 (hw cast wraps, not saturates)


def _build_program(n_tiles=N_TILES):
    nc = bacc.Bacc(
        "TRN2", target_bir_lowering=False, debug=False, enable_asserts=False
    )
    f32 = mybir.dt.float32
    f16 = mybir.dt.float16
    u16 = mybir.dt.uint16
    nq = n_tiles * P
    lhs_a = nc.dram_tensor("lhs_a", (KM, nq), f16, kind="ExternalInput")
    rhs_a = nc.dram_tensor("rhs_a", (KM, N), f16, kind="ExternalInput")
    l_out = nc.dram_tensor("l_out", (nq, UW), u16, kind="ExternalOutput")
    lhs_a_ap, rhs_a_ap = lhs_a.ap(), rhs_a.ap()
    l_ap = l_out.ap()

    with tile.TileContext(nc) as tc:
        with (
            tc.tile_pool(name="const", bufs=1) as cpool,
            tc.tile_pool(name="psum", bufs=2, space="PSUM") as ppool,
            tc.tile_pool(name="csp", bufs=5) as cspool,
            tc.tile_pool(name="t1p", bufs=4) as t1pool,
            tc.tile_pool(name="t2p", bufs=3) as t2pool,
            tc.tile_pool(name="t3p", bufs=4) as t3pool,
            tc.tile_pool(name="outp", bufs=8) as opool,
        ):
            # dependency-free warm-up matmuls that run during the input-DMA
            # prologue (nudges the PE toward its full-rate p-state)
            prime = cpool.tile([KM, 512], f16)
            nc.gpsimd.memset(prime[:, :], 0.0)
            pps = ppool.tile([P, 2048], f32, tag="ps")
            for _ in range(3):
                nc.tensor.matmul(pps[:, :512], prime[:, :128], prime[:, :],
                                 start=True, stop=True)

            # per-512-column-chunk input tiles: the first matmul only waits
            # for its own chunk, not the whole load
            ra_sb = [
                cpool.tile([KM, 512], f16, name=f"ra{j}", tag=f"ra{j}")
                for j in range(8)
            ]
            la_sb = cpool.tile([KM, nq], f16)
            nc.sync.dma_start(la_sb[:, 0:128], lhs_a_ap[:, 0:128])
            nc.sync.dma_start(ra_sb[0][:, :], rhs_a_ap[:, 0:512])
            nc.sync.dma_start(la_sb[:, 128:512], lhs_a_ap[:, 128:512])
            for j in range(1, 8):
                nc.sync.dma_start(ra_sb[j][:, :], rhs_a_ap[:, j * 512 : (j + 1) * 512])
            for j in range(512, nq, 512):
                w = min(512, nq - j)
                nc.sync.dma_start(la_sb[:, j : j + w], lhs_a_ap[:, j : j + w])

            for tp in range(n_tiles // 2):
                # tile pair: L2..L4 merge into one 3D-AP TensorTensor per
                # level (halves the per-level fixed instruction overhead)
                t1 = t1pool.tile([P, 2, 2048], u16, tag="t1")
                for hp in range(2):
                    t = tp * 2 + hp
                    qs = slice(t * P, (t + 1) * P)
                    for h in range(2):
                        ps = ppool.tile([P, 2048], f32, tag="ps")
                        cs = cspool.tile([P, 2048], u16, tag="cs")
                        for j in range(4):
                            cj = h * 4 + j
                            nc.tensor.matmul(
                                ps[:, j * 512 : (j + 1) * 512],
                                la_sb[:, qs], ra_sb[cj][:, :],
                                start=True, stop=True,
                            )
                        nc.scalar.copy(cs[:, :], ps[:, :])
                        nc.vector.tensor_max(
                            t1[:, hp, h * 1024 : (h + 1) * 1024],
                            cs[:, 0:1024], cs[:, 1024:2048],
                        )

                t2 = t2pool.tile([P, 2, 1024], u16, tag="t2")
                nc.vector.tensor_max(
                    t2[:, :, :], t1[:, :, 0:1024], t1[:, :, 1024:2048]
                )
                t3 = t3pool.tile([P, 2, 512], u16, tag="t3")
                nc.vector.tensor_max(
                    t3[:, :, :], t2[:, :, 0:512], t2[:, :, 512:1024]
                )
                t4 = t3pool.tile([P, 2, TOPW], u16, tag="t4")
                nc.vector.tensor_max(
                    t4[:, :, :], t3[:, :, 0:256], t3[:, :, 256:512]
                )

                for hp in range(2):
                    t = tp * 2 + hp
                    u = opool.tile([P, UW], u16, tag="u")
                    l = opool.tile([P, UW], u16, tag="l")
                    for g in range(NG):
                        nc.vector.max(
                            out=u[:, g * 8 : (g + 1) * 8],
                            in_=t4[:, hp, GB3[g] : GB3[g + 1]],
                        )
                    for g in range(NG):
                        nc.vector.max_index(
                            out=l[:, g * 8 : (g + 1) * 8],
                            in_max=u[:, g * 8 : (g + 1) * 8],
                            in_values=t4[:, hp, GB3[g] : GB3[g + 1]],
                        )
                    rs = slice(t * P, (t + 1) * P)
                    nc.sync.dma_start(l_ap[rs, :], l[:])
    nc.compile()
    return nc


_SAMPLE_COLS = np.arange(11, N, 21)[:192]         # 192 fixed probe columns


def _prep_core_inputs(X, core):
    """X: (B, N, C) fp32. Returns input map for one core."""
    b, h = divmod(core, N_CORES // B)
    Xb = X[b]
    xsq = np.sum(Xb * Xb, axis=1, dtype=np.float32)
    ch = Xb.T.astype(np.float16)                  # (C, N)
    half_s = (-(ALPHA0 * 0.5) * xsq).astype(np.float16)
    rhs_a = np.zeros((KM, N), np.float16)
    rhs_a[:C] = ch
    rhs_a[C] = half_s
    rhs_a[C + 1] = half_s
    rhs_a[C + 2 : C + 4] = 1.0

    Q = Xb[h * QROWS : (h + 1) * QROWS]           # (QROWS, C)
    qsq = xsq[h * QROWS : (h + 1) * QROWS]
    # sampled nearest-distance estimate per query (approximate is fine: only
    # resolution depends on it, never clamp-soundness). The diagonal must be
    # masked: a probe column equal to the query itself gives dist 0 and a
    # garbage estimate.
    dprobe = (
        qsq[:, None]
        + xsq[_SAMPLE_COLS][None, :]
        - 2.0 * (Q @ Xb[_SAMPLE_COLS].T)
    )
    qglob = h * QROWS + np.arange(QROWS)
    dprobe[qglob[:, None] == _SAMPLE_COLS[None, :]] = np.inf
    dsamp = dprobe.min(axis=1)
    dist_est = np.maximum(dsamp, (SELF_V - VCENTER) / 1024.0 + 0.1)
    alpha_q = (SELF_V - VCENTER) / dist_est       # per-row scale, <= 1024
    beta = SELF_V - alpha_q * qsq                 # v_self == SELF_V exactly
    qh = ((2.0 * alpha_q)[None, :] * Q.T).astype(np.float16)
    lhs_a = np.zeros((KM, QROWS), np.float16)
    lhs_a[:C] = qh
    lhs_a[C : C + 2] = (alpha_q / ALPHA0).astype(np.float16)
    lhs_a[C + 2 : C + 4] = (0.5 * beta).astype(np.float16)
    return {"lhs_a": lhs_a, "rhs_a": rhs_a}


# slot s -> group g = s//8; T3 position p = GB3[g] + l[s]; columns p + 512k
_GOFF = np.asarray(GB3[:-1], dtype=np.int64)[np.arange(UW) // 8]   # (48,)
_KOFF = (np.arange(STRIDE, dtype=np.int64) * TOPW)                 # (16,)


def _merge_core(L, Xb64, xsq64, q0):
    """L: (R, 48) uint16 local slot indices for queries q0..q0+R-1 of batch b.
    Returns (idx (R,18) int64, flagged-row mask (R,))."""
    R = L.shape[0]
    Ppos = L.astype(np.int64) + _GOFF[None, :]               # (R, 48) in [0,TOPW)
    cols = Ppos[:, :, None] + _KOFF[None, None, :]           # (R, 48, 16)

    # duplicate-slot rule: same T3 position twice within a group
    ps = np.sort(Ppos.reshape(R, NG, 8), axis=2)
    dup = (np.diff(ps, axis=2) == 0).any(axis=(1, 2))

    idx = np.empty((R, K_EFF), np.int64)
    flag = np.empty(R, bool)
    CH = 512
    NC_ = UW * STRIDE
    for c0 in range(0, R, CH):
        c1 = min(c0 + CH, R)
        cc = cols[c0:c1].reshape(c1 - c0, NC_)                # (r, 768)
        # the self column always belongs to the true top-18 (dist 0) but its
        # on-device value wraps mod 2^16 -- inject it unconditionally
        selfc = np.arange(q0 + c0, q0 + c1, dtype=np.int64)[:, None]
        cc = np.concatenate([cc, selfc], axis=1)              # (r, 769)
        g = Xb64[cc]                                          # (r, 769, 64)
        xq = Xb64[q0 + c0 : q0 + c1]                          # (r, 64)
        vals = 2.0 * np.matmul(g, xq[:, :, None])[:, :, 0]    # (r, 769)
        vals -= xsq64[cc]

        # margin rule: per-slot winner, per-group min of the 8 winners
        # (device slots only -- exclude the injected self column)
        w = vals[:, :NC_].reshape(c1 - c0, UW, STRIDE).max(axis=2)
        gmin = w.reshape(c1 - c0, NG, 8).min(axis=2)          # (r, NG)
        t18 = np.partition(vals, vals.shape[1] - K_EFF, axis=1)[
            :, vals.shape[1] - K_EFF
        ]
        flag[c0:c1] = (gmin >= (t18[:, None] - MARGIN)).any(axis=1)

        # the self col may duplicate a device candidate: mask the device copy
        dupself = cc[:, :NC_] == selfc
        vals[:, :NC_][dupself] = -np.inf

        # stable top-18 by (value desc, col asc): sort cols ascending first
        corder = np.argsort(cc, axis=1, kind="stable")
        fc_s = np.take_along_axis(cc, corder, axis=1)
        va_s = np.take_along_axis(vals, corder, axis=1)
        vorder = np.argsort(-va_s, axis=1, kind="stable")[:, :K_EFF]
        idx[c0:c1] = np.take_along_axis(fc_s, vorder, axis=1)
    return idx, (flag | dup)


_NC_CACHE = {}


def kernel(x: np.ndarray) -> np.ndarray:
    x = np.asarray(x)
    assert x.shape == (B, C, N, 1), x.shape
    X = np.ascontiguousarray(np.transpose(x[..., 0], (0, 2, 1)))  # (B, N, C)

    if N_TILES not in _NC_CACHE:
        _NC_CACHE[N_TILES] = _build_program(N_TILES)
    nc = _NC_CACHE[N_TILES]

    in_maps = [_prep_core_inputs(X, c) for c in range(N_CORES)]
    res = run_bass_kernel_spmd(nc, in_maps, core_ids=list(range(N_CORES)))

    X64 = X.astype(np.float64)
    xsq64 = np.einsum("bnc,bnc->bn", X64, X64)

    nn_idx = np.empty((B, N, K_EFF), np.int64)
    bad_rows = [[] for _ in range(B)]
    for core in range(N_CORES):
        b, h = divmod(core, N_CORES // B)
        r = res.results[core]
        idx, bad = _merge_core(r["l_out"], X64[b], xsq64[b], h * QROWS)
        nn_idx[b, h * QROWS : (h + 1) * QROWS] = idx
        if bad.any():
            bad_rows[b].extend((h * QROWS + np.nonzero(bad)[0]).tolist())

    # full exact recompute of flagged rows
    for b in range(B):
        if not bad_rows[b]:
            continue
        rows = np.asarray(sorted(bad_rows[b]))
        S = 2.0 * (X64[b, rows] @ X64[b].T) - xsq64[b][None, :]
        order = np.argsort(-S, axis=1, kind="stable")
        nn_idx[b, rows] = order[:, :K_EFF]

    nn_dil = nn_idx[:, :, ::DILATION]                       # (B, N, 9)
    center = np.broadcast_to(np.arange(N)[None, :, None], nn_dil.shape)
    out = np.stack((nn_dil, center), axis=0).astype(np.int32)
    return out
